# revision 3
# baseline (speedup 1.0000x reference)
"""PatchedVisionExpertMLP TRN2 kernel.

Strategy (8 NeuronCores, SPMD single program):
  - Routing on host: tokens are permuted into [language..., vision...] groups
    (vision = tokens where vmask is True). Each group is padded to a multiple
    of 256 and split into expert-pure token tiles of 512/256.
  - Megatron tensor-parallel over the FFN dim F: each core owns an 11-block
    (1408-col, zero-padded from 11008->11264 total) slice of gate/up columns
    and down rows for BOTH experts. Token tiles choose the expert weights at
    compile time (mask is replicated, so the single SPMD program is valid on
    every core; per-core weight shards differ only as data).
  - Per token-chunk (<=1024 tokens): phase 1 computes g=x@Wg, u=x@Wu per
    f-block (PSUM, fp16 matmuls, fp32 accumulate), h = silu(g)*u into SBUF;
    phase 2 computes partial_out[t,:] = h @ Wd into DRAM.
  - Host unshard: sum the 8 partial outputs (Megatron all-reduce), inverse
    token permutation, reshape.
"""

import numpy as np

import concourse.bacc as bacc
import concourse.tile as tile
import concourse.mybir as mybir
from concourse.bass_utils import run_bass_kernel_spmd

P = 128
KB = 32                 # D / P
D = KB * P              # 4096
FB = 11                 # f-blocks per core
FS = FB * P             # 1408 per-core F shard
NCORES = 8
F_FULL = 11008
F_PAD = FS * NCORES     # 11264
TT = 512                # token tile
CHUNK = 1024            # max tokens per chunk
DT16 = mybir.dt.float16
NDT = 8                 # d-tiles of 512 in phase 2
DTW = 512

VISION_TOKEN_TYPE = 1


def _vision_mask_flat(token_type_ids: np.ndarray) -> np.ndarray:
    tt = np.asarray(token_type_ids)
    is_v = tt == VISION_TOKEN_TYPE
    pair = is_v[:, :-1] & is_v[:, 1:]
    vm = np.concatenate([pair, np.zeros_like(pair[:, :1])], axis=1)
    return vm.reshape(-1)


def _group_tiles(n: int) -> list[int]:
    """Tile sizes (512/256) covering n tokens, padded up to 256-multiples."""
    sizes = [TT] * (n // TT)
    rem = n % TT
    if rem > 256:
        sizes.append(TT)
    elif rem > 0:
        sizes.append(256)
    return sizes


def _plan(nl: int, nv: int):
    """Token tiles [(start, size, expert)] and chunks (lists of tiles)."""
    tiles = []
    pos = 0
    for expert, n in ((0, nl), (1, nv)):
        for sz in _group_tiles(n):
            tiles.append((pos, sz, expert))
            pos += sz
    total = pos
    chunks = []
    cur: list[tuple[int, int, int]] = []
    cur_tok = 0
    for t in tiles:
        if cur and (len(cur) == 2 or cur_tok + t[1] > CHUNK):
            chunks.append(cur)
            cur = []
            cur_tok = 0
        cur.append(t)
        cur_tok += t[1]
    if cur:
        chunks.append(cur)
    return tiles, chunks, total


def _build_nc(chunks, T: int, experts_used, reps: int = 1):
    nc = bacc.Bacc("TRN2", target_bir_lowering=False, debug=False)

    xt = nc.dram_tensor("xt", [P, KB, T], DT16, kind="ExternalInput").ap()
    wg, wu, wd = {}, {}, {}
    for e in experts_used:
        wg[e] = nc.dram_tensor(f"wg{e}", [FB, P, KB, P], DT16, kind="ExternalInput").ap()
        wu[e] = nc.dram_tensor(f"wu{e}", [FB, P, KB, P], DT16, kind="ExternalInput").ap()
        wd[e] = nc.dram_tensor(f"wd{e}", [NDT, P, FB, DTW], DT16, kind="ExternalInput").ap()
    out = nc.dram_tensor("out", [T, D], mybir.dt.float32, kind="ExternalOutput").ap()

    f32 = mybir.dt.float32
    with tile.TileContext(nc) as tc:
        with (
            tc.tile_pool(name="xt", bufs=3) as xt_pool,
            tc.tile_pool(name="h", bufs=1) as h_pool,
            tc.tile_pool(name="wgu", bufs=6) as wgu_pool,
            tc.tile_pool(name="wd", bufs=2) as wd_pool,
            tc.tile_pool(name="sil", bufs=4) as sil_pool,
            tc.tile_pool(name="ob", bufs=4) as ob_pool,
            tc.tile_pool(name="psum", bufs=2, space="PSUM") as psum_pool,
        ):
            ps_tags = ["ps0", "ps1", "ps2", "ps3"]
            ps_rot = 0
            for _rep in range(reps):
                for chunk in chunks:
                    c0 = chunk[0][0]
                    ct = sum(t[1] for t in chunk)
                    # load x chunk (two kb-halves for pipelined prefetch)
                    xa = xt_pool.tile([P, KB // 2, ct], DT16, tag="xt")
                    xb = xt_pool.tile([P, KB // 2, ct], DT16, tag="xt")
                    nc.sync.dma_start(xa[:], xt[:, 0 : KB // 2, c0 : c0 + ct])
                    nc.sync.dma_start(xb[:], xt[:, KB // 2 : KB, c0 : c0 + ct])
                    halves = (xa, xb)

                    h = h_pool.tile([P, FB, ct], DT16, tag="h")

                    # ---- phase 1: g/u matmuls + silu*mul -> h ----
                    for fi in range(FB):
                        slabs = {}
                        for e in sorted({t[2] for t in chunk}):
                            g_sb = wgu_pool.tile([P, KB, P], DT16, tag="wgu")
                            nc.sync.dma_start(g_sb[:], wg[e][fi])
                            u_sb = wgu_pool.tile([P, KB, P], DT16, tag="wgu")
                            nc.sync.dma_start(u_sb[:], wu[e][fi])
                            slabs[e] = (g_sb, u_sb)

                        ps_g, ps_u = [], []
                        for ti, (tstart, tsz, te) in enumerate(chunk):
                            ps_g.append(
                                psum_pool.tile([P, tsz], f32, tag=f"ps{2 * ti}", name=f"psg{ti}")
                            )
                            ps_u.append(
                                psum_pool.tile([P, tsz], f32, tag=f"ps{2 * ti + 1}", name=f"psu{ti}")
                            )
                        for gu in (0, 1):
                            pss = ps_g if gu == 0 else ps_u
                            for kb in range(KB):
                                half = halves[kb // (KB // 2)]
                                ki = kb % (KB // 2)
                                for ti, (tstart, tsz, te) in enumerate(chunk):
                                    lo = tstart - c0
                                    nc.tensor.matmul(
                                        pss[ti][:],
                                        slabs[te][gu][:, kb, :],
                                        half[:, ki, lo : lo + tsz],
                                        start=(kb == 0),
                                        stop=(kb == KB - 1),
                                    )
                        for ti, (tstart, tsz, te) in enumerate(chunk):
                            lo = tstart - c0
                            sil = sil_pool.tile([P, TT], f32, tag="sil")
                            nc.scalar.activation(
                                sil[:, :tsz], ps_g[ti][:], mybir.ActivationFunctionType.Silu
                            )
                            nc.vector.tensor_tensor(
                                h[:, fi, lo : lo + tsz],
                                sil[:, :tsz],
                                ps_u[ti][:],
                                mybir.AluOpType.mult,
                            )

                    # ---- phase 2: out[t, :] = h @ Wd ----
                    for e in sorted({t[2] for t in chunk}):
                        # global token-rows (128-blocks) of this expert in chunk
                        rows = []
                        for tstart, tsz, te in chunk:
                            if te == e:
                                rows.extend(
                                    tstart + b * P for b in range(tsz // P)
                                )
                        for dt_i in range(NDT):
                            wd_sb = wd_pool.tile([P, FB, DTW], DT16, tag="wd")
                            nc.sync.dma_start(wd_sb[:], wd[e][dt_i])
                            for r in rows:
                                lo = r - c0
                                ps_o = psum_pool.tile(
                                    [P, DTW], f32, tag=ps_tags[ps_rot], name="ps_o"
                                )
                                ps_rot = (ps_rot + 1) % 4
                                for fi in range(FB):
                                    nc.tensor.matmul(
                                        ps_o[:],
                                        h[:, fi, lo : lo + P],
                                        wd_sb[:, fi, :],
                                        start=(fi == 0),
                                        stop=(fi == FB - 1),
                                    )
                                ob = ob_pool.tile([P, DTW], f32, tag="ob")
                                nc.vector.tensor_copy(ob[:], ps_o[:])
                                nc.sync.dma_start(
                                    out[r : r + P, dt_i * DTW : (dt_i + 1) * DTW], ob[:]
                                )
    nc.compile()
    return nc


def _prepare(hidden_states, token_type_ids, vision_gate, vision_up, vision_down,
             language_gate, language_up, language_down):
    x = np.asarray(hidden_states, dtype=np.float32).reshape(-1, D)
    vm = _vision_mask_flat(token_type_ids)
    idx_l = np.nonzero(~vm)[0]
    idx_v = np.nonzero(vm)[0]
    nl, nv = len(idx_l), len(idx_v)
    tiles, chunks, T = _plan(nl, nv)
    nl_pad = sum(sz for _, sz, e in tiles if e == 0)

    # permuted + padded tokens, fp16, laid out [p, kb, t]
    xp = np.zeros((T, D), dtype=np.float16)
    xp[:nl] = x[idx_l]
    xp[nl_pad : nl_pad + nv] = x[idx_v]
    xt = np.ascontiguousarray(
        xp.T.reshape(KB, P, T).transpose(1, 0, 2)
    )  # [p, kb, t]

    def gu_layout(w):
        wp = np.zeros((D, F_PAD), dtype=np.float16)
        wp[:, :F_FULL] = np.asarray(w, dtype=np.float32)
        # [kb, p, fbg, f] -> [fbg, p, kb, f]
        return np.ascontiguousarray(
            wp.reshape(KB, P, FB * NCORES, P).transpose(2, 1, 0, 3)
        )

    def d_layout(w):
        wp = np.zeros((F_PAD, D), dtype=np.float16)
        wp[:F_FULL] = np.asarray(w, dtype=np.float32)
        # [fbg, p, dt, dc] -> [dt, p, fbg, dc]
        return np.ascontiguousarray(
            wp.reshape(FB * NCORES, P, NDT, DTW).transpose(2, 1, 0, 3)
        )

    weights = {
        0: (gu_layout(language_gate), gu_layout(language_up), d_layout(language_down)),
        1: (gu_layout(vision_gate), gu_layout(vision_up), d_layout(vision_down)),
    }
    experts_used = sorted({e for _, _, e in tiles})

    in_maps = []
    for c in range(NCORES):
        m = {"xt": xt}
        fb0 = c * FB
        for e in experts_used:
            g, u, d = weights[e]
            m[f"wg{e}"] = g[fb0 : fb0 + FB]
            m[f"wu{e}"] = u[fb0 : fb0 + FB]
            m[f"wd{e}"] = np.ascontiguousarray(d[:, :, fb0 : fb0 + FB, :])
        in_maps.append(m)

    meta = dict(
        chunks=chunks, T=T, experts_used=experts_used,
        idx_l=idx_l, idx_v=idx_v, nl=nl, nv=nv, nl_pad=nl_pad,
        shape=np.asarray(hidden_states).shape,
    )
    return in_maps, meta


def _unshard(results, meta):
    acc = results[0]["out"].astype(np.float32, copy=True)
    for r in results[1:]:
        acc += r["out"]
    y = np.empty((meta["shape"][0] * meta["shape"][1], D), dtype=np.float32)
    y[meta["idx_l"]] = acc[: meta["nl"]]
    y[meta["idx_v"]] = acc[meta["nl_pad"] : meta["nl_pad"] + meta["nv"]]
    return y.reshape(meta["shape"])


def run(reps: int = 1, **inputs):
    in_maps, meta = _prepare(**inputs)
    nc = _build_nc(meta["chunks"], meta["T"], meta["experts_used"], reps=reps)
    res = run_bass_kernel_spmd(nc, in_maps, core_ids=list(range(NCORES)))
    return _unshard(res.results, meta)


def kernel(**inputs) -> np.ndarray:
    return run(reps=1, **inputs)
